# revision 1
# baseline (speedup 1.0000x reference)
"""Multi-head causal attention on 8 Trainium2 NeuronCores.

Sharding: 8 cores = 4 batches x 2 head-halves.  Each core computes, for one
batch, 8 of the 16 heads end-to-end (QKV projection with column-sharded
weights, causal attention, and a partial output projection with row-sharded
Wo).  The host sums the two partial outputs per batch and adds bo.

All matmuls run in float32r (full PE rate, ~1e-4 matmul rel err).  Host
pre-transposes the activations so no on-chip transposes are needed:

  qT/kT  [feat, seq]   = (W[:, cols].T @ X.T) accumulated over D chunks
  scoresT[k, q]        = kT_h.T-slice vs qT_h-slice matmul (K = DH = 64)
  expT                 = ACT exp (scale=1/sqrt(DH)) straight off PSUM
  ctxT_aug[65, q]      = v_aug.T @ expT  (col 64 of v_aug is ones -> row 64
                         of ctxT is the softmax denominator)
  out[q, :]            = sum over feat chunks of ctxT-normalized.T @ Wo

The two heads of each 128-feature chunk are emitted interleaved in phase B so
their K=64 score matmuls land in different PE row groups (base partitions 0
and 64) and overlap, and so ACT exp of one head pipelines with PE matmuls of
the other.
"""

import sys

if "/opt/trn_rl_repo" not in sys.path:
    sys.path.insert(0, "/opt/trn_rl_repo")

import numpy as np

import concourse.bass as bass  # noqa: F401
import concourse.tile as tile
from concourse import bacc, mybir

F32 = mybir.dt.float32
F32R = mybir.dt.float32r

# Problem shape (hardcoded per the harness contract).
B, S, D, H = 4, 2048, 1024, 16
DH = D // H
N_CORES = 8
HC = H // 2              # heads per core
DC = HC * DH             # feature columns per core (512)
QB = 512                 # query block (free dim of scoresT matmuls)
KB = 128                 # key block (partition dim of scoresT)
RB = 512                 # row block for q/k projections (moving free dim)


def _build_core_kernel(mask_mode: str, mm_dt="f32r", reps: int = 1,
                       phases: str = "ABC", b_ilv: bool = True,
                       s_bufs: int = 1, ctx_bufs: int = 2, g_kb: int = 2,
                       recip_fast: bool = False,
                       norm_mode: str = "div", tri_on: bool = True,
                       exp_dup: int = 1, tri_mm: bool = False,
                       copy_eng: str = "act", halfwork: bool = False,
                       hwdge_in: bool = True):
    """mask_mode: 'causal' | 'dense' | 'general'. reps>1 repeats the whole
    computation in-NEFF (for timing: amortizes host<->device transfer)."""
    if isinstance(mm_dt, str):
        mm_dt = {"f32r": F32R, "bf16": mybir.dt.bfloat16,
                 "f32": F32}[mm_dt]
    nc = bacc.Bacc("TRN2", target_bir_lowering=False, debug=False)

    in_dt = mm_dt if hwdge_in else F32
    xq_d = nc.dram_tensor("xqT", [D, S], in_dt, kind="ExternalInput")
    xk_d = nc.dram_tensor("xkT", [D, S], in_dt, kind="ExternalInput")
    xv_d = nc.dram_tensor("xvT", [D, S], in_dt, kind="ExternalInput")
    wq_d = nc.dram_tensor("wq", [D, DC], in_dt, kind="ExternalInput")
    wk_d = nc.dram_tensor("wk", [D, DC], in_dt, kind="ExternalInput")
    wv_d = nc.dram_tensor("wv", [D, DC], in_dt, kind="ExternalInput")
    wo_d = nc.dram_tensor("wo", [DC, D], in_dt, kind="ExternalInput")
    ldma = nc.sync.dma_start if hwdge_in else nc.gpsimd.dma_start
    if mask_mode == "general":
        # host passes mask[0,0].T * -1e9, shape [S(k), S(q)]
        mneg_d = nc.dram_tensor("maskTneg", [S, S], F32, kind="ExternalInput")
    out_d = nc.dram_tensor("out", [S, D], F32, kind="ExternalOutput")

    n_kc = D // 128          # contraction chunks for projections (8)
    n_ch = DC // 128         # feature chunks per core (4); 2 heads per chunk
    n_rb = S // RB           # q/k projection row blocks
    n_qb = S // QB           # query blocks (4)
    n_kb = S // KB           # key blocks (16)
    kb_per_qb = QB // KB     # diag kblocks per query block (4)

    inv_sqrt_dh = 1.0 / float(np.sqrt(DH))

    def _copy(dst, src_ap):
        if copy_eng == "dve":
            nc.vector.tensor_copy(dst, src_ap)
        else:
            nc.scalar.activation(dst, src_ap,
                                 mybir.ActivationFunctionType.Copy)

    xq_r = xq_d.ap().rearrange("(c p) s -> p c s", p=128)
    xk_r = xk_d.ap().rearrange("(c p) s -> p c s", p=128)
    xv_r = xv_d.ap().rearrange("(c p) s -> p c s", p=128)

    with tile.TileContext(nc) as tc:
      for _rep in range(reps):
        with (
            tc.tile_pool(name="res", bufs=1) as res,
            tc.tile_pool(name="small", bufs=1) as small,
        ):
            # ---- constants ------------------------------------------------
            tri = small.tile([KB, KB], F32, tag="tri")
            nc.gpsimd.memset(tri[:], 0.0)
            ones_c = small.tile([128, HC], F32, tag="ones_c")
            nc.gpsimd.memset(ones_c[:], 1.0)
            eye_r = small.tile([KB, KB], mm_dt, tag="eye_r")
            tri_r = small.tile([KB, KB], mm_dt, tag="tri_r")
            if mask_mode != "dense":
                # scoresT[k, q]: keep where q - k >= 0, else -1e9
                nc.gpsimd.affine_select(
                    out=tri[:], in_=tri[:],
                    compare_op=mybir.AluOpType.is_ge,
                    fill=-1e9, base=0,
                    pattern=[[1, KB]], channel_multiplier=-1,
                )
                eye_f = small.tile([KB, KB], F32, tag="eye_f")
                nc.gpsimd.memset(eye_f[:], 0.0)
                nc.gpsimd.affine_select(
                    out=eye_f[:], in_=eye_f[:],
                    compare_op=mybir.AluOpType.not_equal,
                    fill=1.0, base=0,
                    pattern=[[-1, KB]], channel_multiplier=1,
                )
                nc.vector.tensor_copy(eye_r[:], eye_f[:])
                nc.vector.tensor_copy(tri_r[:], tri[:])

            # ---- resident tensors ----------------------------------------
            qT = [res.tile([128, S], mm_dt, tag=f"qT{c}", name=f"qT{c}")
                  for c in range(n_ch)]
            kT = [res.tile([128, S], mm_dt, tag=f"kT{c}", name=f"kT{c}")
                  for c in range(n_ch)]
            # v_aug: per 128-row block, per head, 64 value cols + ones col
            v_aug = [res.tile([128, HC, DH + 1], mm_dt, tag=f"v{r}",
                              name=f"v{r}")
                     for r in range(n_kb)]

            # ---- phase A: QKV projections --------------------------------
            with (
                tc.tile_pool(name="wa", bufs=1) as wa,
                tc.tile_pool(name="xs", bufs=3) as xs,
                tc.tile_pool(name="aps", bufs=2, space="PSUM") as aps,
                tc.tile_pool(name="aqk", bufs=3, space="PSUM") as aqk,
            ):
                wv_t = wa.tile([128, n_kc, DC], mm_dt, tag="wv")
                wq_t = wa.tile([128, n_kc, DC], mm_dt, tag="wq")
                wk_t = wa.tile([128, n_kc, DC], mm_dt, tag="wk")
                ldma(
                    wv_t[:], wv_d.ap().rearrange("(c p) n -> p c n", p=128))
                ldma(
                    wq_t[:], wq_d.ap().rearrange("(c p) n -> p c n", p=128))
                ldma(
                    wk_t[:], wk_d.ap().rearrange("(c p) n -> p c n", p=128))

                # V projection: two 128-row blocks per load
                for r2 in range(n_kb // 2):
                    xv_t = xs.tile([128, n_kc, 2 * KB], mm_dt, tag="x")
                    ldma(
                        xv_t[:], xv_r[:, :, 2 * r2 * KB:(2 * r2 + 2) * KB])
                    for rr in range(2):
                        r = 2 * r2 + rr
                        ps = aps.tile([128, DC], F32, tag="pv")
                        for kc in range(n_kc):
                            nc.tensor.matmul(
                                ps[:], xv_t[:, kc, rr * KB:(rr + 1) * KB],
                                wv_t[:, kc, :],
                                start=(kc == 0), stop=(kc == n_kc - 1))
                        nc.vector.tensor_copy(v_aug[r][:, :, DH], ones_c[:])
                        _copy(v_aug[r][:, :, 0:DH],
                              ps[:].rearrange("p (h d) -> p h d", h=HC))

                # Q/K projections -> transposed layout [feat, seq]
                for r in range(n_rb):
                    xq_t = xs.tile([128, n_kc, RB], mm_dt, tag="x")
                    xk_t = xs.tile([128, n_kc, RB], mm_dt, tag="x")
                    ldma(
                        xq_t[:], xq_r[:, :, r * RB:(r + 1) * RB])
                    ldma(
                        xk_t[:], xk_r[:, :, r * RB:(r + 1) * RB])
                    for c in range(n_ch):
                        psq = aqk.tile([128, RB], F32, tag="pq")
                        psk = aqk.tile([128, RB], F32, tag="pk")
                        for kc in range(n_kc):
                            nc.tensor.matmul(
                                psq[:], wq_t[:, kc, c * 128:(c + 1) * 128],
                                xq_t[:, kc, :],
                                start=(kc == 0), stop=(kc == n_kc - 1))
                        for kc in range(n_kc):
                            nc.tensor.matmul(
                                psk[:], wk_t[:, kc, c * 128:(c + 1) * 128],
                                xk_t[:, kc, :],
                                start=(kc == 0), stop=(kc == n_kc - 1))
                        _copy(qT[c][:, r * RB:(r + 1) * RB], psq[:])
                        _copy(kT[c][:, r * RB:(r + 1) * RB], psk[:])

            # ---- phase B: attention (two heads interleaved) --------------
            with tc.tile_pool(name="cw", bufs=1) as cw:
              ctxT = [cw.tile([128, S], mm_dt, tag=f"ctxT{c}",
                              name=f"ctxT{c}") for c in range(n_ch)]
              with (
                tc.tile_pool(name="bex", bufs=2) as bex,
                tc.tile_pool(name="bse", bufs=4) as bse,
                tc.tile_pool(name="bps", bufs=s_bufs, space="PSUM") as bps,
                tc.tile_pool(name="bctx", bufs=ctx_bufs, space="PSUM") as bctx,
              ):
                # wo load overlaps phase B (reuses the zone wv/wq/wk held)
                wo_t = cw.tile([128, n_ch, D], mm_dt, tag="wo")
                ldma(
                    wo_t[:], wo_d.ap().rearrange("(c p) n -> p c n", p=128))

                def diag_j(kb, qb):
                    if mask_mode == "causal" and kb >= qb * kb_per_qb:
                        return kb - qb * kb_per_qb
                    return -1

                for c in range(n_ch if "B" in phases else 0):
                    all_heads = [
                        {"h": 2 * c + half, "base": half * 64,
                         "tag": half if b_ilv else 0}
                        for half in range(2)
                    ]
                    head_groups = ([all_heads] if b_ilv
                                   else [[h] for h in all_heads])
                    for heads in head_groups:
                      for qb in range(n_qb):
                        q0 = qb * QB
                        kmax = ((qb + 1) * kb_per_qb
                                if mask_mode == "causal" else n_kb)
                        if halfwork:   # timing probe only (wrong results)
                            kmax = max(2, kmax // 2)
                        for hd in heads:
                            hd["psc"] = bctx.tile(
                                [DH + 1, QB], F32, tag=f"pctx{hd['tag']}",
                                name=f"psc{hd['tag']}")
                        n_g = (kmax + g_kb - 1) // g_kb
                        for g in range(n_g):
                            kbs = [kb for kb in range(g_kb * g,
                                                      g_kb * (g + 1))
                                   if kb < kmax]
                            for hd in heads:
                                hd["pss"] = bps.tile(
                                    [128, g_kb * QB], F32,
                                    tag=f"s{hd['tag']}",
                                    name=f"pss{hd['tag']}")
                                hd["ext"] = bex.tile(
                                    [128, g_kb * QB], mm_dt,
                                    tag=f"e{hd['tag']}",
                                    name=f"ext{hd['tag']}")
                            # score matmuls, head-interleaved
                            for i, kb in enumerate(kbs):
                                j = diag_j(kb, qb)
                                off = j * KB if j > 0 else 0
                                use_trimm = (j >= 0 and tri_on
                                             and tri_mm)
                                for hd in heads:
                                    b0 = hd["base"]  # noqa
                                    nc.tensor.matmul(
                                        hd["pss"][:, i * QB + off:
                                                  (i + 1) * QB],
                                        kT[c][b0:b0 + 64,
                                              kb * KB:(kb + 1) * KB],
                                        qT[c][b0:b0 + 64, q0 + off:q0 + QB],
                                        start=True, stop=not use_trimm,
                                        skip_group_check=use_trimm)
                                    if use_trimm:
                                        # accumulate -1e9 strict-lower mask
                                        # onto the diagonal 128x128 block
                                        nc.tensor.matmul(
                                            hd["pss"][:, i * QB + off:
                                                      i * QB + off + KB],
                                            eye_r[:], tri_r[:],
                                            start=False, stop=True,
                                            skip_group_check=True)
                                if j >= 0 and tri_on and not tri_mm:
                                    for hd in heads:
                                        nc.vector.tensor_tensor(
                                            hd["pss"][:, i * QB + off:
                                                      i * QB + off + KB],
                                            hd["pss"][:, i * QB + off:
                                                      i * QB + off + KB],
                                            tri[:], op=mybir.AluOpType.add)
                                if mask_mode == "general":
                                    for hd in heads:
                                        mng = bse.tile([128, QB], F32,
                                                       tag="mng")
                                        nc.sync.dma_start(
                                            mng[:],
                                            mneg_d.ap()
                                            [kb * KB:(kb + 1) * KB,
                                             q0:q0 + QB])
                                        nc.vector.tensor_tensor(
                                            hd["pss"][:, i * QB:
                                                      (i + 1) * QB],
                                            hd["pss"][:, i * QB:
                                                      (i + 1) * QB],
                                            mng[:], op=mybir.AluOpType.add)
                            # exp
                            whole = (len(kbs) == g_kb
                                     and all(diag_j(kb, qb) < 0
                                             for kb in kbs))
                            for hd in heads:
                                for _dup in range(exp_dup - 1):
                                    scr = bex.tile(
                                        [128, g_kb * QB], mm_dt,
                                        tag=f"scr{hd['tag']}",
                                        name=f"scr{hd['tag']}")
                                    nc.scalar.activation(
                                        scr[:], hd["pss"][:],
                                        mybir.ActivationFunctionType.Exp,
                                        scale=inv_sqrt_dh)
                                if whole:
                                    nc.scalar.activation(
                                        hd["ext"][:], hd["pss"][:],
                                        mybir.ActivationFunctionType.Exp,
                                        scale=inv_sqrt_dh)
                                else:
                                    for i, kb in enumerate(kbs):
                                        j = diag_j(kb, qb)
                                        off = j * KB if j > 0 else 0
                                        nc.scalar.activation(
                                            hd["ext"][:, i * QB + off:
                                                      (i + 1) * QB],
                                            hd["pss"][:, i * QB + off:
                                                      (i + 1) * QB],
                                            mybir.ActivationFunctionType.Exp,
                                            scale=inv_sqrt_dh)
                            # ctx accumulation
                            for i, kb in enumerate(kbs):
                                j = diag_j(kb, qb)
                                off = j * KB if j > 0 else 0
                                for hd in heads:
                                    nc.tensor.matmul(
                                        hd["psc"][:, off:QB],
                                        v_aug[kb][:, hd["h"], :],
                                        hd["ext"][:, i * QB + off:
                                                  (i + 1) * QB],
                                        start=(kb == 0),
                                        stop=(kb == kmax - 1))
                        # normalize: ctxT = psc[0:64] * (1/psc[64]) bcast
                        for hd in heads:
                            b0 = hd["base"]
                            if norm_mode == "copy":   # timing probe only
                                nc.vector.tensor_copy(
                                    ctxT[c][b0:b0 + 64, q0:q0 + QB],
                                    hd["psc"][0:DH, :])
                                continue
                            se_r = bse.tile([1, QB], F32, tag="ser")
                            se_b = bse.tile([64, QB], F32, tag="seb")
                            if recip_fast:
                                nc.vector.reciprocal_approx_fast(
                                    out=se_r[:], in_=hd["psc"][DH:DH + 1, :])
                            else:
                                nc.vector.reciprocal(
                                    se_r[:], hd["psc"][DH:DH + 1, :])
                            nc.gpsimd.partition_broadcast(se_b[:], se_r[:])
                            nc.vector.tensor_tensor(
                                ctxT[c][b0:b0 + 64, q0:q0 + QB],
                                hd["psc"][0:DH, :], se_b[:],
                                op=mybir.AluOpType.mult)

              # ---- phase C: output projection ----------------------------
              with (
                  tc.tile_pool(name="cout", bufs=2) as cout,
                  tc.tile_pool(name="cps", bufs=2, space="PSUM") as cps,
              ):
                  for r2 in range(n_kb // 2 if "C" in phases else 0):
                      ot = cout.tile([128, 2, D], F32, tag="ot")
                      for rr in range(2):
                          r = 2 * r2 + rr
                          pos = [cps.tile([128, 512], F32, tag=f"po{oc}",
                                          name=f"po{oc}")
                                 for oc in range(D // 512)]
                          for c in range(n_ch):
                              for oc in range(D // 512):
                                  nc.tensor.matmul(
                                      pos[oc][:],
                                      ctxT[c][:, r * KB:(r + 1) * KB],
                                      wo_t[:, c, oc * 512:(oc + 1) * 512],
                                      start=(c == 0), stop=(c == n_ch - 1))
                          for oc in range(D // 512):
                              _copy(ot[:, rr, oc * 512:(oc + 1) * 512],
                                    pos[oc][:])
                      nc.sync.dma_start(
                          out_d.ap().rearrange("(r2 rr p) n -> p r2 rr n",
                                               rr=2, p=128)[:, r2],
                          ot[:])

    nc.compile()
    return nc


_KERNEL_CACHE = {}


def _get_kernel(mask_mode):
    if mask_mode not in _KERNEL_CACHE:
        _KERNEL_CACHE[mask_mode] = _build_core_kernel(mask_mode)
    return _KERNEL_CACHE[mask_mode]


def _classify_mask(mask):
    m = np.asarray(mask).reshape(S, S)
    if not m.any():
        return "dense"
    iu = np.triu_indices(S, 1)
    causal = np.zeros((S, S), np.float32)
    causal[iu] = 1.0
    if np.array_equal(m, causal):
        return "causal"
    return "general"


def np_f32r(a):
    """Round fp32 array to float32r (RNE, drop low 12 mantissa bits)."""
    b = np.ascontiguousarray(a, dtype=np.float32).view(np.uint32).astype(np.uint64)
    low = b & 0xFFF
    out = b & ~np.uint64(0xFFF)
    inc = (low > 0x800) | ((low == 0x800) & ((b >> 12) & 1).astype(bool))
    return (out + inc * 0x1000).astype(np.uint64).astype(
        np.uint32).view(np.float32)


def make_in_maps(queries, keys, values, mask, Wq, bq, Wk, bk, Wv, bv, Wo, bo):
    mask_mode = _classify_mask(mask)
    assert not np.any(bq) and not np.any(bk) and not np.any(bv), (
        "nonzero qkv biases not supported by this kernel build")
    in_maps = []
    for core in range(N_CORES):
        b, half = divmod(core, 2)
        cols = slice(half * DC, (half + 1) * DC)
        m = {
            "xqT": np_f32r(np.asarray(queries)[b].T),
            "xkT": np_f32r(np.asarray(keys)[b].T),
            "xvT": np_f32r(np.asarray(values)[b].T),
            "wq": np_f32r(np.asarray(Wq)[:, cols]),
            "wk": np_f32r(np.asarray(Wk)[:, cols]),
            "wv": np_f32r(np.asarray(Wv)[:, cols]),
            "wo": np_f32r(np.asarray(Wo)[half * DC:(half + 1) * DC, :]),
        }
        if mask_mode == "general":
            m["maskTneg"] = np.ascontiguousarray(
                np.asarray(mask).reshape(S, S).T * np.float32(-1e9))
        in_maps.append(m)
    return mask_mode, in_maps


def combine_results(results, bo):
    out = np.empty((B, S, D), np.float32)
    for b in range(B):
        out[b] = results[2 * b]["out"] + results[2 * b + 1]["out"]
    out += np.asarray(bo).reshape(1, 1, D).astype(np.float32)
    return out


def kernel(queries, keys, values, mask, Wq, bq, Wk, bk, Wv, bv, Wo, bo):
    from concourse import bass_utils

    mask_mode, in_maps = make_in_maps(
        queries, keys, values, mask, Wq, bq, Wk, bk, Wv, bv, Wo, bo)
    nc = _get_kernel(mask_mode)
    res = bass_utils.run_bass_kernel_spmd(
        nc, in_maps, core_ids=list(range(N_CORES)), trace=False)
    return combine_results(res.results, np.asarray(bo))

